# revision 29
# baseline (speedup 1.0000x reference)
"""Trainium2 Bass kernel for nn_CADenseMul.

Math (see reference):
    chi  = sigmoid(context @ W + Bc)          # [B, R]
    s    = S * chi                            # [B, R]
    out  = ((inputs @ U) * s) @ V.T + bias    # [B, UNITS]

Strategy:
  - Data-parallel over batch B across 8 cores (B=4096 -> 512 rows/core).
  - Host-side prep (not device time): per-core transposed activation shards
    packed into SBUF-layout blobs ([128, cols] contiguous per partition ->
    line-rate DMA); fold S into U (U_s = U * S); ship V pre-transposed;
    cast streams to bf16.
  - Device (transposed-activation layout, batch as the free dim):
        h.T    = W.T @ ctx.T          (PSUM; sigmoid+Bc on ACT)
        proj.T = U_s.T @ x.T          (256-wide b-slices, k-half chunks)
        psT    = proj.T * chi.T       (DVE, cast bf16)
        out    = psT.T @ V.T          (psT stationary, natural-layout out)
  - Schedule: loads stream on the sync HWDGE ring in the order PE consumes
    them (ub_a, x0a, wc, ub_b, x0b, x1a, x1b, vb0, vb1) so proj starts
    ~12us; the chi chain (wc -> h -> sigmoid) rides mid-stream, its h
    matmuls slotted into the PE gap between proj chunks; finals run as
    unit-half passes (q01 needs only vb0, q23 only vb1).  Scalar engine
    carries no DMA triggers (sigmoid table preloaded during warm-up);
    stores split across the sync + gpsimd rings.
  - Output stored bf16 (halves store traffic); host concats, adds bias fp32.
"""

import os
import numpy as np
import ml_dtypes

import concourse.bass as bass
import concourse.tile as tile
from concourse import bacc, mybir
from concourse.bass_utils import run_bass_kernel_spmd

N_CORES = 8
B, D_IN, D_CTX, UNITS, R = 4096, 2048, 512, 2048, 256
BS = B // N_CORES        # 512 batch rows per core
KT_X = D_IN // 128       # 16
KT_C = D_CTX // 128      # 4
RT = R // 128            # 2
NBT = BS // 128          # 4 output batch tiles
PW = 256                 # proj slice width (batch cols per proj pass)
NP = BS // PW            # 2 proj slices

ACT_DTYPE = os.environ.get("CAD_DTYPE", "bf16")    # bf16 | f32r
OUT_BF16 = os.environ.get("CAD_OUT", "bf16") == "bf16"
N_WARM = int(os.environ.get("CAD_WARM", "115"))    # warm-up matmuls
N_WARM2 = int(os.environ.get("CAD_WARM2", "12"))   # gap-filler matmuls

_COMPILED = {}


def _build(key):
    act_dtype, out_bf16, n_warm, n_warm2 = key
    dt_act = mybir.dt.bfloat16 if act_dtype == "bf16" else mybir.dt.float32r
    dt_f32 = mybir.dt.float32
    dt_out = mybir.dt.bfloat16 if out_bf16 else dt_f32

    nc = bacc.Bacc("TRN2", target_bir_lowering=False, debug=False,
                   num_devices=N_CORES)

    # packed blobs: [128, cols] per-partition-contiguous
    wc = nc.dram_tensor("wc", [128, KT_C * R + KT_C * BS], dt_act,
                        kind="ExternalInput").ap()          # W | ctxT
    ub = nc.dram_tensor("ub", [128, KT_X * R], dt_act,
                        kind="ExternalInput").ap()          # U_s, col=k*R+r
    xh = [nc.dram_tensor(f"x{i}", [128, KT_X * PW], dt_act,
                         kind="ExternalInput").ap() for i in range(NP)]
    vb = nc.dram_tensor("vb", [128, RT * UNITS], dt_act,
                        kind="ExternalInput").ap()          # uh*2048+rh*1024+uu
    Bc2 = nc.dram_tensor("Bc2", [128, RT], dt_f32, kind="ExternalInput").ap()
    out = nc.dram_tensor("out", [BS, UNITS], dt_out, kind="ExternalOutput").ap()
    dummy_out = nc.dram_tensor("dummy_out", [128, 16], dt_f32,
                               kind="ExternalOutput").ap()

    UH = KT_X * R // 2       # ub half-cols (k 0-7 | k 8-15)
    XH = KT_X * PW // 2      # x blob half-cols (k 0-7 | k 8-15)
    VH = RT * UNITS // 2     # vb half-cols (units 0-1023 | 1024-2047)

    with tile.TileContext(nc) as tc:
        with (
            tc.tile_pool(name="consts", bufs=1) as consts,
            tc.tile_pool(name="osb", bufs=NBT) as osb,
            tc.tile_pool(name="ps_p", bufs=2, space="PSUM") as ps_p,
            tc.tile_pool(name="ps_o", bufs=6, space="PSUM") as ps_o,
        ):
            warm_sb = consts.tile([128, 128], dt_act, tag="warm")
            nc.vector.memset(warm_sb[:], 0.0)

            # ---- loads: sync ring, in PE consumption order ----
            ub_sb = consts.tile([128, KT_X * R], dt_act, tag="ub")
            x_sb = [consts.tile([128, KT_X * PW], dt_act, tag=f"x{i}",
                                name=f"x_sb{i}") for i in range(NP)]
            wc_sb = consts.tile([128, KT_C * R + KT_C * BS], dt_act, tag="wc")
            vb_sb = consts.tile([128, RT * UNITS], dt_act, tag="vb")
            Bc_sb = consts.tile([128, RT], dt_f32, tag="bc")

            XQ = KT_X * PW // 4
            nc.sync.dma_start(ub_sb[:, :UH], ub[:, :UH])
            nc.sync.dma_start(x_sb[0][:, :XH], xh[0][:, :XH])
            nc.gpsimd.dma_start(Bc_sb[:], Bc2[:])
            nc.sync.dma_start(ub_sb[:, UH:], ub[:, UH:])
            nc.sync.dma_start(x_sb[0][:, XH:], xh[0][:, XH:])
            nc.sync.dma_start(wc_sb[:], wc[:])
            nc.sync.dma_start(vb_sb[:, :VH], vb[:, :VH])
            nc.sync.dma_start(vb_sb[:, VH:], vb[:, VH:])
            nc.sync.dma_start(x_sb[1][:, :XH], xh[1][:, :XH])
            # split the x tail so its completion receipts overlap
            nc.sync.dma_start(x_sb[1][:, XH:XH + XQ], xh[1][:, XH:XH + XQ])
            nc.sync.dma_start(x_sb[1][:, XH + XQ:], xh[1][:, XH + XQ:])

            # ---- PE warm-up: garbage matmuls, no data deps; the warm PSUM
            # lives in its own pool so its keepalive copy never gates the
            # proj PSUM slot rotation ----
            warm_ps = ps_o.tile([128, 512], dt_f32, tag="ops",
                                name="warm_ps")
            for _ in range(max(n_warm, 1)):
                nc.tensor.matmul(warm_ps[:, :64], warm_sb[:], warm_sb[:, :64],
                                 start=True, stop=True)
            warm_sink = consts.tile([128, 16], dt_f32, tag="warm_sink")
            nc.vector.tensor_copy(warm_sink[:, :8], warm_ps[:, :8])
            # (the copy above also releases warm_ps's ps_o slot early)
            # preload the ACT sigmoid table while ACT is idle (reads warm_sb,
            # NOT warm_ps -- must not wait on the matmuls)
            nc.scalar.activation(warm_sink[:, 8:16], warm_sb[:, :8],
                                 mybir.ActivationFunctionType.Sigmoid)

            W_off = 0
            ctx_off = KT_C * R
            chi_sb = consts.tile([128, RT * BS], dt_f32, tag="chi")
            psT_sb = consts.tile([128, RT * BS], dt_act, tag="psT")
            p_ps = {}

            def emit_proj(i, kh):
                """proj slice i (b cols i*PW..), k-half kh of the reduction."""
                for rh in range(RT):
                    if kh == 0:
                        p_ps[rh] = ps_p.tile([128, PW], dt_f32, tag="pps",
                                             name=f"p_ps{i}{rh}")
                    ps = p_ps[rh]
                    for k in range(kh * 8, kh * 8 + 8):
                        nc.tensor.matmul(
                            ps[:],
                            ub_sb[:, k * R + rh * 128: k * R + rh * 128 + 128],
                            x_sb[i][:, k * PW: (k + 1) * PW],
                            start=(k == 0), stop=(k == KT_X - 1))

            def emit_mul(i):
                for rh in range(RT):
                    nc.vector.tensor_mul(
                        psT_sb[:, rh * BS + i * PW: rh * BS + i * PW + PW],
                        p_ps[rh][:],
                        chi_sb[:, rh * BS + i * PW: rh * BS + i * PW + PW])

            def emit_h():
                # h PSUMs ride the ps_o pool (consumed by sigmoid well before
                # the final passes recycle the slots)
                for rh in range(RT):
                    ps = ps_o.tile([128, 512], dt_f32, tag="ops",
                                   name=f"h_ps{rh}")
                    for n in range(KT_C):
                        nc.tensor.matmul(
                            ps[:],
                            wc_sb[:, W_off + n * R + rh * 128:
                                     W_off + n * R + rh * 128 + 128],
                            wc_sb[:, ctx_off + n * BS:
                                     ctx_off + (n + 1) * BS],
                            start=(n == 0), stop=(n == KT_C - 1))
                    nc.scalar.activation(
                        chi_sb[:, rh * BS:(rh + 1) * BS], ps[:],
                        mybir.ActivationFunctionType.Sigmoid,
                        bias=Bc_sb[:, rh:rh + 1])

            o_tiles = [osb.tile([128, UNITS], dt_out, tag=f"o{t}",
                                name=f"o_sb{t}") for t in range(NBT)]
            copy_rr = [0]

            def emit_copy(dst, ps):
                # alternate PSUM->SBUF copies over DVE + ACT (the only
                # engines with PSUM read access)
                i = copy_rr[0] % 2
                copy_rr[0] += 1
                if i == 0:
                    nc.vector.tensor_copy(dst, ps[:])
                else:
                    nc.scalar.activation(
                        dst, ps[:], mybir.ActivationFunctionType.Copy)

            def emit_final(t, half):
                # (tile, unit-half) pass: 2 q-chunks, each 2 matmuls + one
                # [128,512] PSUM->SBUF copy (alternating DVE/ACT); one store
                o_sb = o_tiles[t]
                for qq in range(2):
                    q = half * 2 + qq
                    ps = ps_o.tile([128, 512], dt_f32, tag="ops")
                    for rh in range(RT):
                        nc.tensor.matmul(
                            ps[:],
                            psT_sb[:, rh * BS + t * 128:
                                      rh * BS + t * 128 + 128],
                            vb_sb[:, half * VH + rh * 1024 + qq * 512:
                                     half * VH + rh * 1024 + qq * 512 + 512],
                            start=(rh == 0), stop=(rh == RT - 1))
                    emit_copy(o_sb[:, q * 512:(q + 1) * 512], ps)
                nc.sync.dma_start(
                    out[t * 128:(t + 1) * 128,
                        half * 1024:(half + 1) * 1024],
                    o_sb[:, half * 1024:(half + 1) * 1024])

            # ---- PE pipeline: p0a | h | warm2 | p0b | p1a | p1b | fa | fb --
            emit_proj(0, 0)
            emit_h()
            # flush the DCE-keepalive early so its completion receipt doesn't
            # extend the kernel-exit drain
            nc.gpsimd.dma_start(dummy_out[:], warm_sink[:])

            emit_proj(0, 1)
            emit_mul(0)
            # slice-0 finals (tiles 0,1) run while x1 still streams
            for t in range(NP):
                emit_final(t, 0)
            for t in range(NP):
                emit_final(t, 1)
            emit_proj(1, 0)
            emit_proj(1, 1)
            emit_mul(1)
            for t in range(NP, NBT):
                emit_final(t, 0)
            for t in range(NP, NBT):
                emit_final(t, 1)

    nc.compile()
    return nc


def _get_nc(key):
    if key not in _COMPILED:
        _COMPILED[key] = _build(key)
    return _COMPILED[key]


def _pack(a, p=128):
    """[n*p, m] row-major -> [p, n*m]: partition p holds rows p, p+128, ..."""
    n = a.shape[0] // p
    return np.ascontiguousarray(
        a.reshape(n, p, a.shape[1]).transpose(1, 0, 2).reshape(p, -1))


def _prep_in_maps(inputs, context, U, S, V, W, Bc, act_dtype):
    np_act = ml_dtypes.bfloat16 if act_dtype == "bf16" else np.float32

    Us = np.asarray(U, np.float32) * np.asarray(S, np.float32)[None, :]
    ub = _pack(Us).astype(np_act)
    # vb repacked units-half-major: col = uh*2048 + rh*1024 + uu
    vb = _pack(np.ascontiguousarray(np.asarray(V, np.float32).T))
    vb = np.ascontiguousarray(
        vb.reshape(128, RT, 2, UNITS // 2).transpose(0, 2, 1, 3)
          .reshape(128, RT * UNITS)).astype(np_act)
    W32 = np.asarray(W, np.float32)
    Bc2 = np.ascontiguousarray(
        np.asarray(Bc, np.float32).reshape(RT, 128).T)

    x = np.asarray(inputs, np.float32)
    ctx = np.asarray(context, np.float32)
    in_maps = []
    for c in range(N_CORES):
        ctxT = ctx[c * BS:(c + 1) * BS, :].T
        wcb = np.concatenate([_pack(W32), _pack(np.ascontiguousarray(ctxT))],
                             axis=1).astype(np_act)
        xT = x[c * BS:(c + 1) * BS, :].T
        m = {"wc": wcb, "ub": ub, "vb": vb, "Bc2": Bc2}
        for i in range(NP):
            m[f"x{i}"] = _pack(np.ascontiguousarray(
                xT[:, i * PW:(i + 1) * PW])).astype(np_act)
        in_maps.append(m)
    return in_maps


def kernel(inputs, context, U, S, V, W, Bc, bias, _run_kwargs=None):
    key = (ACT_DTYPE, OUT_BF16, N_WARM, N_WARM2)
    nc = _get_nc(key)
    in_maps = _prep_in_maps(inputs, context, U, S, V, W, Bc, ACT_DTYPE)
    res = run_bass_kernel_spmd(nc, in_maps, list(range(N_CORES)),
                               **(_run_kwargs or {}))
    if _run_kwargs:
        kernel.last_results = res
    out = np.concatenate([np.asarray(res.results[c]["out"]).astype(np.float32)
                          for c in range(N_CORES)], axis=0)
    out += np.asarray(bias, np.float32)[None, :]
    return out


# revision 30
# speedup vs baseline: 1.0328x; 1.0328x over previous
"""Trainium2 Bass kernel for nn_CADenseMul.

Math (see reference):
    chi  = sigmoid(context @ W + Bc)          # [B, R]
    s    = S * chi                            # [B, R]
    out  = ((inputs @ U) * s) @ V.T + bias    # [B, UNITS]

Strategy:
  - Data-parallel over batch B across 8 cores (B=4096 -> 512 rows/core).
  - Host-side prep (not device time): per-core transposed activation shards
    packed into SBUF-layout blobs ([128, cols] contiguous per partition ->
    line-rate DMA); fold S into U (U_s = U * S); ship V pre-transposed;
    cast streams to bf16.
  - Device (transposed-activation layout, batch as the free dim):
        h.T    = W.T @ ctx.T          (PSUM; sigmoid+Bc on ACT)
        proj.T = U_s.T @ x.T          (256-wide b-slices, k-half chunks)
        psT    = proj.T * chi.T       (DVE, cast bf16)
        out    = psT.T @ V.T          (psT stationary, natural-layout out)
  - Schedule: loads stream on the sync HWDGE ring in the order PE consumes
    them (ub_a, x0a, wc, ub_b, x0b, x1a, x1b, vb0, vb1) so proj starts
    ~12us; the chi chain (wc -> h -> sigmoid) rides mid-stream, its h
    matmuls slotted into the PE gap between proj chunks; finals run as
    unit-half passes (q01 needs only vb0, q23 only vb1).  Scalar engine
    carries no DMA triggers (sigmoid table preloaded during warm-up);
    stores split across the sync + gpsimd rings.
  - Output stored bf16 (halves store traffic); host concats, adds bias fp32.
"""

import os
import numpy as np
import ml_dtypes

import concourse.bass as bass
import concourse.tile as tile
from concourse import bacc, mybir
from concourse.bass_utils import run_bass_kernel_spmd

N_CORES = 8
B, D_IN, D_CTX, UNITS, R = 4096, 2048, 512, 2048, 256
BS = B // N_CORES        # 512 batch rows per core
KT_X = D_IN // 128       # 16
KT_C = D_CTX // 128      # 4
RT = R // 128            # 2
NBT = BS // 128          # 4 output batch tiles
PW = 256                 # proj slice width (batch cols per proj pass)
NP = BS // PW            # 2 proj slices

ACT_DTYPE = os.environ.get("CAD_DTYPE", "bf16")    # bf16 | f32r
OUT_BF16 = os.environ.get("CAD_OUT", "bf16") == "bf16"
N_WARM = int(os.environ.get("CAD_WARM", "115"))    # warm-up matmuls
N_WARM2 = int(os.environ.get("CAD_WARM2", "12"))   # gap-filler matmuls

_COMPILED = {}


def _build(key):
    act_dtype, out_bf16, n_warm, n_warm2 = key
    dt_act = mybir.dt.bfloat16 if act_dtype == "bf16" else mybir.dt.float32r
    dt_f32 = mybir.dt.float32
    dt_out = mybir.dt.bfloat16 if out_bf16 else dt_f32

    nc = bacc.Bacc("TRN2", target_bir_lowering=False, debug=False,
                   num_devices=N_CORES)

    # packed blobs: [128, cols] per-partition-contiguous
    wc = nc.dram_tensor("wc", [128, KT_C * R + KT_C * BS], dt_act,
                        kind="ExternalInput").ap()          # W | ctxT
    ub = nc.dram_tensor("ub", [128, KT_X * R], dt_act,
                        kind="ExternalInput").ap()          # U_s, col=k*R+r
    xh = [nc.dram_tensor(f"x{i}", [128, KT_X * PW], dt_act,
                         kind="ExternalInput").ap() for i in range(NP)]
    vb = nc.dram_tensor("vb", [128, RT * UNITS], dt_act,
                        kind="ExternalInput").ap()          # uh*2048+rh*1024+uu
    Bc2 = nc.dram_tensor("Bc2", [128, RT], dt_f32, kind="ExternalInput").ap()
    out = nc.dram_tensor("out", [BS, UNITS], dt_out, kind="ExternalOutput").ap()
    dummy_out = nc.dram_tensor("dummy_out", [128, 16], dt_f32,
                               kind="ExternalOutput").ap()

    UH = KT_X * R // 2       # ub half-cols (k 0-7 | k 8-15)
    XH = KT_X * PW // 2      # x blob half-cols (k 0-7 | k 8-15)
    VH = RT * UNITS // 2     # vb half-cols (units 0-1023 | 1024-2047)

    with tile.TileContext(nc) as tc:
        with (
            tc.tile_pool(name="consts", bufs=1) as consts,
            tc.tile_pool(name="osb", bufs=NBT) as osb,
            tc.tile_pool(name="ps_p", bufs=2, space="PSUM") as ps_p,
            tc.tile_pool(name="ps_o", bufs=6, space="PSUM") as ps_o,
        ):
            warm_sb = consts.tile([128, 128], dt_act, tag="warm")
            nc.vector.memset(warm_sb[:], 0.0)

            # ---- loads: sync ring, in PE consumption order ----
            ub_sb = consts.tile([128, KT_X * R], dt_act, tag="ub")
            x_sb = [consts.tile([128, KT_X * PW], dt_act, tag=f"x{i}",
                                name=f"x_sb{i}") for i in range(NP)]
            wc_sb = consts.tile([128, KT_C * R + KT_C * BS], dt_act, tag="wc")
            vb_sb = consts.tile([128, RT * UNITS], dt_act, tag="vb")
            Bc_sb = consts.tile([128, RT], dt_f32, tag="bc")

            XQ = KT_X * PW // 4
            nc.sync.dma_start(ub_sb[:, :UH], ub[:, :UH])
            nc.sync.dma_start(x_sb[0][:, :XH], xh[0][:, :XH])
            nc.gpsimd.dma_start(Bc_sb[:], Bc2[:])
            nc.sync.dma_start(ub_sb[:, UH:], ub[:, UH:])
            nc.sync.dma_start(x_sb[0][:, XH:], xh[0][:, XH:])
            nc.sync.dma_start(wc_sb[:], wc[:])
            nc.sync.dma_start(x_sb[1][:, :XH], xh[1][:, :XH])
            # split the x tail so its completion receipts overlap
            nc.sync.dma_start(x_sb[1][:, XH:XH + XQ], xh[1][:, XH:XH + XQ])
            nc.sync.dma_start(x_sb[1][:, XH + XQ:], xh[1][:, XH + XQ:])
            nc.sync.dma_start(vb_sb[:, :VH], vb[:, :VH])
            nc.sync.dma_start(vb_sb[:, VH:], vb[:, VH:])

            # ---- PE warm-up: garbage matmuls, no data deps; the warm PSUM
            # lives in its own pool so its keepalive copy never gates the
            # proj PSUM slot rotation ----
            warm_ps = ps_o.tile([128, 512], dt_f32, tag="ops",
                                name="warm_ps")
            for _ in range(max(n_warm, 1)):
                nc.tensor.matmul(warm_ps[:, :64], warm_sb[:], warm_sb[:, :64],
                                 start=True, stop=True)
            warm_sink = consts.tile([128, 16], dt_f32, tag="warm_sink")
            nc.vector.tensor_copy(warm_sink[:, :8], warm_ps[:, :8])
            # (the copy above also releases warm_ps's ps_o slot early)
            # preload the ACT sigmoid table while ACT is idle (reads warm_sb,
            # NOT warm_ps -- must not wait on the matmuls)
            nc.scalar.activation(warm_sink[:, 8:16], warm_sb[:, :8],
                                 mybir.ActivationFunctionType.Sigmoid)

            W_off = 0
            ctx_off = KT_C * R
            chi_sb = consts.tile([128, RT * BS], dt_f32, tag="chi")
            psT_sb = consts.tile([128, RT * BS], dt_act, tag="psT")
            p_ps = {}

            def emit_proj(i, kh):
                """proj slice i (b cols i*PW..), k-half kh of the reduction."""
                for rh in range(RT):
                    if kh == 0:
                        p_ps[rh] = ps_p.tile([128, PW], dt_f32, tag="pps",
                                             name=f"p_ps{i}{rh}")
                    ps = p_ps[rh]
                    for k in range(kh * 8, kh * 8 + 8):
                        nc.tensor.matmul(
                            ps[:],
                            ub_sb[:, k * R + rh * 128: k * R + rh * 128 + 128],
                            x_sb[i][:, k * PW: (k + 1) * PW],
                            start=(k == 0), stop=(k == KT_X - 1))

            def emit_mul(i):
                for rh in range(RT):
                    nc.vector.tensor_mul(
                        psT_sb[:, rh * BS + i * PW: rh * BS + i * PW + PW],
                        p_ps[rh][:],
                        chi_sb[:, rh * BS + i * PW: rh * BS + i * PW + PW])

            def emit_h():
                # h PSUMs ride the ps_o pool (consumed by sigmoid well before
                # the final passes recycle the slots)
                for rh in range(RT):
                    ps = ps_o.tile([128, 512], dt_f32, tag="ops",
                                   name=f"h_ps{rh}")
                    for n in range(KT_C):
                        nc.tensor.matmul(
                            ps[:],
                            wc_sb[:, W_off + n * R + rh * 128:
                                     W_off + n * R + rh * 128 + 128],
                            wc_sb[:, ctx_off + n * BS:
                                     ctx_off + (n + 1) * BS],
                            start=(n == 0), stop=(n == KT_C - 1))
                    nc.scalar.activation(
                        chi_sb[:, rh * BS:(rh + 1) * BS], ps[:],
                        mybir.ActivationFunctionType.Sigmoid,
                        bias=Bc_sb[:, rh:rh + 1])

            o_tiles = [osb.tile([128, UNITS], dt_out, tag=f"o{t}",
                                name=f"o_sb{t}") for t in range(NBT)]
            copy_rr = [0]

            def emit_copy(dst, ps):
                # alternate PSUM->SBUF copies over DVE + ACT (the only
                # engines with PSUM read access)
                i = copy_rr[0] % 2
                copy_rr[0] += 1
                if i == 0:
                    nc.vector.tensor_copy(dst, ps[:])
                else:
                    nc.scalar.activation(
                        dst, ps[:], mybir.ActivationFunctionType.Copy)

            def emit_final(t, half):
                # (tile, unit-half) pass: 2 q-chunks, each 2 matmuls + one
                # [128,512] PSUM->SBUF copy (alternating DVE/ACT); one store
                o_sb = o_tiles[t]
                for qq in range(2):
                    q = half * 2 + qq
                    ps = ps_o.tile([128, 512], dt_f32, tag="ops")
                    for rh in range(RT):
                        nc.tensor.matmul(
                            ps[:],
                            psT_sb[:, rh * BS + t * 128:
                                      rh * BS + t * 128 + 128],
                            vb_sb[:, half * VH + rh * 1024 + qq * 512:
                                     half * VH + rh * 1024 + qq * 512 + 512],
                            start=(rh == 0), stop=(rh == RT - 1))
                    emit_copy(o_sb[:, q * 512:(q + 1) * 512], ps)
                nc.sync.dma_start(
                    out[t * 128:(t + 1) * 128,
                        half * 1024:(half + 1) * 1024],
                    o_sb[:, half * 1024:(half + 1) * 1024])

            # ---- PE pipeline: p0a | h | warm2 | p0b | p1a | p1b | fa | fb --
            emit_proj(0, 0)
            emit_h()
            # flush the DCE-keepalive early so its completion receipt doesn't
            # extend the kernel-exit drain
            nc.gpsimd.dma_start(dummy_out[:], warm_sink[:])

            emit_proj(0, 1)
            emit_mul(0)
            emit_proj(1, 0)
            emit_proj(1, 1)
            emit_mul(1)
            for t in range(NBT):
                emit_final(t, 0)
            for t in range(NBT):
                emit_final(t, 1)

    nc.compile()
    return nc


def _get_nc(key):
    if key not in _COMPILED:
        _COMPILED[key] = _build(key)
    return _COMPILED[key]


def _pack(a, p=128):
    """[n*p, m] row-major -> [p, n*m]: partition p holds rows p, p+128, ..."""
    n = a.shape[0] // p
    return np.ascontiguousarray(
        a.reshape(n, p, a.shape[1]).transpose(1, 0, 2).reshape(p, -1))


def _prep_in_maps(inputs, context, U, S, V, W, Bc, act_dtype):
    np_act = ml_dtypes.bfloat16 if act_dtype == "bf16" else np.float32

    Us = np.asarray(U, np.float32) * np.asarray(S, np.float32)[None, :]
    ub = _pack(Us).astype(np_act)
    # vb repacked units-half-major: col = uh*2048 + rh*1024 + uu
    vb = _pack(np.ascontiguousarray(np.asarray(V, np.float32).T))
    vb = np.ascontiguousarray(
        vb.reshape(128, RT, 2, UNITS // 2).transpose(0, 2, 1, 3)
          .reshape(128, RT * UNITS)).astype(np_act)
    W32 = np.asarray(W, np.float32)
    Bc2 = np.ascontiguousarray(
        np.asarray(Bc, np.float32).reshape(RT, 128).T)

    x = np.asarray(inputs, np.float32)
    ctx = np.asarray(context, np.float32)
    in_maps = []
    for c in range(N_CORES):
        ctxT = ctx[c * BS:(c + 1) * BS, :].T
        wcb = np.concatenate([_pack(W32), _pack(np.ascontiguousarray(ctxT))],
                             axis=1).astype(np_act)
        xT = x[c * BS:(c + 1) * BS, :].T
        m = {"wc": wcb, "ub": ub, "vb": vb, "Bc2": Bc2}
        for i in range(NP):
            m[f"x{i}"] = _pack(np.ascontiguousarray(
                xT[:, i * PW:(i + 1) * PW])).astype(np_act)
        in_maps.append(m)
    return in_maps


def kernel(inputs, context, U, S, V, W, Bc, bias, _run_kwargs=None):
    key = (ACT_DTYPE, OUT_BF16, N_WARM, N_WARM2)
    nc = _get_nc(key)
    in_maps = _prep_in_maps(inputs, context, U, S, V, W, Bc, ACT_DTYPE)
    res = run_bass_kernel_spmd(nc, in_maps, list(range(N_CORES)),
                               **(_run_kwargs or {}))
    if _run_kwargs:
        kernel.last_results = res
    out = np.concatenate([np.asarray(res.results[c]["out"]).astype(np.float32)
                          for c in range(N_CORES)], axis=0)
    out += np.asarray(bias, np.float32)[None, :]
    return out
